# revision 21
# baseline (speedup 1.0000x reference)
# Mistral sliding-window attention (B=1, S=2048, H=4096, 32 q heads / 8 kv
# heads, window 4096 -> plain causal at this S) on 8 Trainium2 NeuronCores.
#
# Sharding: tensor-parallel over heads. Core c owns q heads 4c..4c+3 and kv
# head c; hidden_states replicated (host-transposed to [H, S] bf16).
#
# v3 design (dense-PE pipeline, bf16 data, stationary-reuse everywhere):
# the PE pays ~50ns extra per matmul whenever the stationary operand
# changes, so every loop is ordered to run back-to-back matmuls with the
# same stationary:
# - Phase A (QKV projection): chunk PAIRS - each weight slice w[k][:, m] is
#   stationary for two matmuls (token chunks 2cp and 2cp+1) accumulating in
#   [128, 2(chunk), 512] psum tags; m-halves {q0,q1,q2} / {q3,k,v} keep the
#   pair within 6 psum banks. x is streamed twice (DMA has headroom). ACT
#   copies psum->bf16 staging, RoPE on DVE during the next half, V tiles
#   PE-transposed off the critical path.
# - Phase B attention: per chunk c and HEAD PAIR, waves over kv tiles j:
#   kT[j] is stationary for both heads' score matmuls, V[j] for both PV
#   matmuls, ones for both denominator matmuls; exp on ACT in [128,2,512]
#   pt tiles (bf16), causal staircase on GpSimd, PV lag-6 queue so the PE
#   never waits on exp or the head-boundary normalize.
# - o_proj per token block g: column-chunk PAIRS - each ao tile is
#   stationary for two matmuls (columns 2ccp, 2ccp+1); all 4 heads
#   accumulate in PSUM across all 8 banks; outputs cast to bf16 (ACT+DVE
#   alternating), DMA'd to per-block partial tensors; ReduceScatter(add)
#   per block overlaps the next chunk's attention; final out copies ride
#   the gpsimd queue behind their RS.

from contextlib import ExitStack

import numpy as np
import ml_dtypes

import concourse.bacc as bacc
import concourse.bass as bass
import concourse.mybir as mybir
import concourse.tile as tile
from concourse.bass_utils import run_bass_kernel_spmd
from concourse.masks import make_identity

HIDDEN = 4096
NH = 32
NKV = 8
HD = 128
THETA = 10000.0
S = 2048
NCORES = 8

QH = NH // NCORES          # 4 q heads per core
DQ = QH * HD               # 512 (per-core q/attn width)
DOUT = DQ + 2 * HD         # 768 = q heads + k + v projection width
KT = HIDDEN // 128         # 32 contraction tiles
KG = 8                     # k-tiles per x DMA / inner k-group
TCH = 512                  # token chunk (matmul moving dim)
NTCH = S // TCH            # 4
KVT = S // 128             # 16 kv tiles
SCALE = 1.0 / float(np.sqrt(HD))

F32 = mybir.dt.float32
BF16 = mybir.dt.bfloat16
EXP = mybir.ActivationFunctionType.Exp
NPBF16 = ml_dtypes.bfloat16


def _rope(nc, rp, stg_half, qdst, cs2, sn2):
    """RoPE one [128, 512] head-tile: stg (bf16 SBUF) -> qdst (bf16 SBUF).

    qdst = stg*cs2 + rotate_half(stg)*sn2, with cs2 = [cos; cos] and
    sn2 = [-sin; sin] stacked on 128 partitions (host-precomputed), so all
    DVE ops are partition-aligned; the rotate is two SBUF->SBUF DMAs.
    """
    b = rp.tile([128, TCH], BF16, name="rope_b")
    nc.sync.dma_start(out=b[0:64, :], in_=stg_half[64:128, :])
    nc.sync.dma_start(out=b[64:128, :], in_=stg_half[0:64, :])
    ta = rp.tile([128, TCH], BF16, name="rope_t")
    tb = rp.tile([128, TCH], BF16, name="rope_u")
    nc.vector.tensor_mul(ta, stg_half, cs2)
    nc.vector.tensor_mul(tb, b, sn2)
    nc.vector.tensor_add(qdst, ta, tb)


def build_kernel_body(ctx: ExitStack, tc: tile.TileContext, outs, ins):
    nc = tc.nc
    xT, wqkv, ow, cos_t, sin_t, stair = (
        ins["xT"], ins["wqkv"], ins["ow"], ins["cos_t"], ins["sin_t"], ins["stair"],
    )
    out = outs["out"]

    # one partial tensor per token block so RS(g) never false-serializes
    # against the o_proj writes of block g+1
    partials = [nc.dram_tensor(f"partial{g}", [TCH, HIDDEN], BF16).ap()
                for g in range(NTCH)]
    rs_out = nc.dram_tensor("rs_out", [NTCH, S // NTCH // NCORES, HIDDEN],
                            BF16).ap()
    p3s = [p.rearrange("(b p) d -> p b d", p=128) for p in partials]

    singles = ctx.enter_context(tc.tile_pool(name="singles", bufs=1))
    # persistent bf16 state
    qT = singles.tile([128, QH, S], BF16)     # roped q, head h -> qT[:, h, :]
    kT = singles.tile([128, S], BF16)         # roped k
    V = singles.tile([128, KVT, HD], BF16)    # V[:, j, :] = [tok 128, d 128]
    ao = singles.tile([128, QH, S], BF16)     # attention out per head
    ones_sb = singles.tile([128, 128], BF16)
    ident_sb = singles.tile([128, 128], BF16)
    stair_sb = singles.tile([128, 896], BF16)
    cos_sb = singles.tile([128, S], BF16)
    sin_sb = singles.tile([128, S], BF16)
    ow_sb = singles.tile([128, QH, HIDDEN], BF16)   # o_w rows, d on partition

    # shared PSUM tags: four [128, 2, 512] f32 accumulators (8 banks)
    psh = ctx.enter_context(tc.tile_pool(name="psh", bufs=1, space="PSUM"))

    def T(i):
        return psh.tile([128, 2, TCH], F32, name=f"T{i}", tag=f"T{i}")

    # ---- DMA front matter ------------------------------------------------
    wq3 = wqkv.rearrange("(k p) d -> p k d", p=128)
    x3 = xT.rearrange("(k p) s -> p k s", p=128)

    with (
        tc.tile_pool(name="wq", bufs=1) as wp,
        tc.tile_pool(name="xt", bufs=6) as xp,
        tc.tile_pool(name="stg", bufs=2) as sp,
        tc.tile_pool(name="rope", bufs=3) as rp,
        tc.tile_pool(name="pva", bufs=1, space="PSUM") as ppv,
    ):
        # chunk-pair 0 supply: x tiles for chunks 0 and 1 interleaved with
        # the weight k-groups so k-ordered consumption is never starved
        w_sb = [wp.tile([128, DOUT], BF16, name=f"w{k}", tag=f"w{k}")
                for k in range(KT)]
        xg_cp0 = [[None] * (KT // KG) for _ in range(2)]
        for kg in range(KT // KG):
            for cc in range(2):
                xg = xp.tile([128, KG, TCH], BF16, name="xg")
                if kg == 0 and cc == 0:
                    nc.sync.dma_start(out=xg[:, 0:2, :], in_=x3[:, 0:2, 0:TCH])
                    nc.sync.dma_start(out=xg[:, 2:KG, :],
                                      in_=x3[:, 2:KG, 0:TCH])
                else:
                    nc.sync.dma_start(
                        out=xg, in_=x3[:, kg * KG:(kg + 1) * KG,
                                       cc * TCH:(cc + 1) * TCH])
                xg_cp0[cc][kg] = xg
            for k in range(kg * KG, (kg + 1) * KG):
                nc.sync.dma_start(out=w_sb[k], in_=wq3[:, k, :])
        nc.sync.dma_start(out=cos_sb, in_=cos_t)
        nc.sync.dma_start(out=sin_sb, in_=sin_t)
        nc.sync.dma_start(out=stair_sb, in_=stair)
        nc.vector.memset(ones_sb, 1.0)
        make_identity(nc, ident_sb)

        # ---- phase A: QKV projection (chunk pairs) + RoPE ----------------
        # queue of (kind, args) work from finished halves, drained into the
        # PE/ACT/DVE stream while later halves run
        pend = []

        def drain_pend(budget):
            n = 0
            while pend and n < budget:
                kind, args = pend.pop(0)
                if kind == "rope":
                    stg_half, dst, cs, sn = args
                    _rope(nc, rp, stg_half, dst, cs, sn)
                else:  # V transpose (PE, cheap, late-emitted)
                    stg_half, j = args
                    pvt = ppv.tile([128, 128], BF16, name="pv", tag="pv")
                    nc.tensor.transpose(pvt, stg_half, ident_sb)
                    nc.scalar.copy(out=V[:, j, :], in_=pvt)
                n += 1

        for cp in range(2):
            c0, c1 = 2 * cp, 2 * cp + 1
            for half in range(2):
                pss = [T(i) for i in range(3)]
                for kg in range(KT // KG):
                    xgs = []
                    for ci, c in enumerate((c0, c1)):
                        if cp == 0 and half == 0:
                            xg = xg_cp0[ci][kg]
                        else:
                            xg = xp.tile([128, KG, TCH], BF16, name="xg")
                            nc.sync.dma_start(
                                out=xg, in_=x3[:, kg * KG:(kg + 1) * KG,
                                               c * TCH:(c + 1) * TCH])
                        xgs.append(xg)
                    for ki in range(KG):
                        k = kg * KG + ki
                        for mi in range(3):
                            m = 3 * half + mi
                            for ci in range(2):
                                nc.tensor.matmul(
                                    pss[mi][:, ci, :],
                                    lhsT=w_sb[k][:, m * 128:(m + 1) * 128],
                                    rhs=xgs[ci][:, ki, :],
                                    start=(k == 0), stop=(k == KT - 1),
                                )
                    drain_pend(3 if kg > 0 else 0)
                # psum -> bf16 staging; split across ACT and DVE so the
                # tags free within ~1.5us for the next half
                stgs = []
                for mi in range(3):
                    stg = sp.tile([128, 2, TCH], BF16, name=f"stg{mi}")
                    if mi == 1:
                        nc.vector.tensor_copy(stg, pss[mi])
                    else:
                        nc.scalar.copy(out=stg, in_=pss[mi])
                    stgs.append(stg)
                # queue rope / V-transpose work for this half
                for ci, c in enumerate((c0, c1)):
                    lo = c * TCH
                    cs = cos_sb[:, lo:lo + TCH]
                    sn = sin_sb[:, lo:lo + TCH]
                    if half == 0:   # q heads 0,1,2
                        for mi in range(3):
                            pend.append(("rope", (stgs[mi][:, ci, :],
                                                  qT[:, mi, lo:lo + TCH],
                                                  cs, sn)))
                    else:           # q head 3, k, v
                        pend.append(("rope", (stgs[0][:, ci, :],
                                              qT[:, 3, lo:lo + TCH], cs, sn)))
                        pend.append(("rope", (stgs[1][:, ci, :],
                                              kT[:, lo:lo + TCH], cs, sn)))
                        for t in range(4):
                            j = 4 * c + t
                            pend.append(("vt", (
                                stgs[2][:, ci, t * 128:(t + 1) * 128], j)))
            if cp == 0:
                # o_proj weights: 4.2MB, queued after pair-0 supply so it
                # loads during pair 1 without starving the projection
                nc.sync.dma_start(out=ow_sb, in_=ow)
        drain_pend(10 ** 9)

    # ---- phase B: attention + o_proj + ReduceScatter ---------------------
    with (
        tc.tile_pool(name="pt", bufs=8) as ptp,
        tc.tile_pool(name="nrm", bufs=2) as nrmp,
        tc.tile_pool(name="ost", bufs=8) as ostp,
        tc.tile_pool(name="psb", bufs=1, space="PSUM") as psb,
    ):
        def T3():
            return psb.tile([128, 2, TCH], F32, name="T3", tag="T3")
        def attention_hpair(hp, c, pvq):
            """Attention for heads (2hp, 2hp+1), q chunk c. Score/PV/ones
            stationaries (kT[j], V[j], ones) are each shared by the two
            heads' back-to-back matmuls."""
            h0, h1 = 2 * hp, 2 * hp + 1
            jmax = 4 * c + 3
            tpo = T(2)   # po planes for the two heads
            tps = T3()   # ps planes for the two heads
            q0 = qT[:, h0, c * TCH:(c + 1) * TCH]
            q1 = qT[:, h1, c * TCH:(c + 1) * TCH]

            def pv_wave(args):
                pt, j = args
                nc.tensor.matmul(tpo[:, 0, :], lhsT=V[:, j, :],
                                 rhs=pt[:, 0, :],
                                 start=(j == 0), stop=(j == jmax))
                nc.tensor.matmul(tpo[:, 1, :], lhsT=V[:, j, :],
                                 rhs=pt[:, 1, :],
                                 start=(j == 0), stop=(j == jmax))
                nc.tensor.matmul(tps[:, 0, :], lhsT=ones_sb,
                                 rhs=pt[:, 0, :],
                                 start=(j == 0), stop=(j == jmax))
                nc.tensor.matmul(tps[:, 1, :], lhsT=ones_sb,
                                 rhs=pt[:, 1, :],
                                 start=(j == 0), stop=(j == jmax))

            for j in range(jmax + 1):
                sct = T(j % 2)
                nc.tensor.matmul(sct[:, 0, :],
                                 lhsT=kT[:, j * 128:(j + 1) * 128],
                                 rhs=q0, start=True, stop=True)
                nc.tensor.matmul(sct[:, 1, :],
                                 lhsT=kT[:, j * 128:(j + 1) * 128],
                                 rhs=q1, start=True, stop=True)
                pt = ptp.tile([128, 2, TCH], BF16, name="pt")
                nc.scalar.activation(pt, sct, EXP, scale=SCALE)
                rdiag = j - 4 * c
                if rdiag >= 0:  # tile touches the causal diagonal
                    off = 384 - rdiag * 128
                    for i in range(2):
                        nc.gpsimd.tensor_mul(
                            pt[:, i, :], pt[:, i, :],
                            stair_sb[:, off:off + TCH])
                pvq.append((pv_wave, (pt, j), None))
                if len(pvq) > 6:
                    fn, args, fin = pvq.pop(0)
                    fn(args)
                    if fin is not None:
                        fin()

            def finalize(hp=hp, c=c, tpo=tpo, tps=tps):
                rec = nrmp.tile([128, 2, TCH], F32, name="rec")
                nc.vector.reciprocal(rec, tps)
                for i, h in enumerate((2 * hp, 2 * hp + 1)):
                    nc.vector.tensor_mul(
                        ao[:, h, c * TCH:(c + 1) * TCH],
                        tpo[:, i, :], rec[:, i, :])
            fn, args, fin = pvq[-1]
            pvq[-1] = (fn, args, finalize)

        def drain_pvq(pvq):
            while pvq:
                fn, args, fin = pvq.pop(0)
                fn(args)
                if fin is not None:
                    fin()

        def oproj_chunk(g):
            """partials[g] = local 4-head o_proj for tokens [g*512,(g+1)*512).
            Column pairs: each ao tile is stationary for two matmuls."""
            for ccp in range(HIDDEN // TCH // 2):
                cA, cB = 2 * ccp, 2 * ccp + 1
                acc = [T(0), T(1), T(2), T3()]  # [ccA: t01,t23][ccB: t01,t23]
                for h in range(QH):
                    for t in range(4):
                        lhs = ao[:, h, g * TCH + t * 128:g * TCH + (t + 1) * 128]
                        nc.tensor.matmul(
                            acc[t // 2][:, t % 2, :], lhsT=lhs,
                            rhs=ow_sb[:, h, cA * TCH:(cA + 1) * TCH],
                            start=(h == 0), stop=(h == QH - 1))
                        nc.tensor.matmul(
                            acc[2 + t // 2][:, t % 2, :], lhsT=lhs,
                            rhs=ow_sb[:, h, cB * TCH:(cB + 1) * TCH],
                            start=(h == 0), stop=(h == QH - 1))
                for q in range(4):
                    cc = cA if q < 2 else cB
                    ost = ostp.tile([128, 2, TCH], BF16, name="ost")
                    if q % 2 == 0:
                        nc.scalar.copy(out=ost, in_=acc[q])
                    else:
                        nc.vector.tensor_copy(ost, acc[q])
                    nc.sync.dma_start(
                        out=p3s[g][:, 2 * (q % 2):2 * (q % 2) + 2,
                                   cc * TCH:(cc + 1) * TCH],
                        in_=ost)

        for c in range(NTCH):
            pvq = []
            for hp in range(2):
                attention_hpair(hp, c, pvq)
            drain_pvq(pvq)
            oproj_chunk(c)
            nc.gpsimd.collective_compute(
                "ReduceScatter",
                mybir.AluOpType.add,
                ins=[partials[c][:, :]],
                outs=[rs_out[c]],
                replica_groups=[list(range(NCORES))],
            )
            # on the gpsimd queue (behind RS(c)) so it cannot head-of-line
            # block the sync-queue tile DMAs
            nc.gpsimd.dma_start(out=out[c], in_=rs_out[c])


_NC_CACHE = None


def build_program():
    global _NC_CACHE
    if _NC_CACHE is not None:
        return _NC_CACHE
    nc = bacc.Bacc("TRN2", target_bir_lowering=False, debug=False,
                   num_devices=NCORES)
    ins = {
        "xT": nc.dram_tensor("xT", [HIDDEN, S], BF16, kind="ExternalInput").ap(),
        "wqkv": nc.dram_tensor("wqkv", [HIDDEN, DOUT], BF16,
                               kind="ExternalInput").ap(),
        "ow": nc.dram_tensor("ow", [128, QH, HIDDEN], BF16,
                             kind="ExternalInput").ap(),
        "cos_t": nc.dram_tensor("cos_t", [128, S], BF16,
                                kind="ExternalInput").ap(),
        "sin_t": nc.dram_tensor("sin_t", [128, S], BF16,
                                kind="ExternalInput").ap(),
        "stair": nc.dram_tensor("stair", [128, 896], BF16,
                                kind="ExternalInput").ap(),
    }
    outs = {"out": nc.dram_tensor(
        "out", [NTCH, S // NTCH // NCORES, HIDDEN], BF16,
        kind="ExternalOutput").ap()}
    with tile.TileContext(nc) as tc:
        with ExitStack() as ctx:
            build_kernel_body(ctx, tc, outs, ins)
    nc.compile()
    _NC_CACHE = nc
    return nc


def make_in_maps(hidden_states, position_ids, q_w, k_w, v_w, o_w):
    x = np.asarray(hidden_states, dtype=np.float32).reshape(S, HIDDEN)
    xT = np.ascontiguousarray(x.T).astype(NPBF16)
    pos = np.asarray(position_ids).reshape(S).astype(np.float64)
    inv = 1.0 / (THETA ** (np.arange(0, HD, 2, dtype=np.float64) / HD))
    fr = inv[:, None] * pos[None, :]                       # [64, S]
    cos_t = np.concatenate([np.cos(fr), np.cos(fr)], 0).astype(NPBF16)
    sin_t = np.concatenate([-np.sin(fr), np.sin(fr)], 0).astype(NPBF16)
    u = np.arange(896, dtype=np.int64)[None, :]
    kvi = np.arange(128, dtype=np.int64)[:, None]
    stair = ((u - kvi) >= 384).astype(NPBF16)              # [128, 896]

    q_w = np.asarray(q_w, dtype=np.float32)
    k_w = np.asarray(k_w, dtype=np.float32)
    v_w = np.asarray(v_w, dtype=np.float32)
    o_w = np.asarray(o_w, dtype=np.float32)

    in_maps = []
    for c in range(NCORES):
        wqkv = np.ascontiguousarray(np.concatenate(
            [q_w[:, c * DQ:(c + 1) * DQ],
             k_w[:, c * HD:(c + 1) * HD],
             v_w[:, c * HD:(c + 1) * HD]], axis=1)).astype(NPBF16)
        owc = np.ascontiguousarray(
            o_w[c * DQ:(c + 1) * DQ, :].reshape(QH, 128, HIDDEN)
            .transpose(1, 0, 2)).astype(NPBF16)
        in_maps.append({"xT": xT, "wqkv": wqkv, "ow": owc,
                        "cos_t": cos_t, "sin_t": sin_t, "stair": stair})
    return in_maps


def assemble_output(outs_per_core):
    """outs_per_core[c] = [NTCH, 64, HIDDEN] bf16; stitch to [1, S, HIDDEN]."""
    rows = S // NTCH // NCORES
    full = np.empty((S, HIDDEN), dtype=np.float32)
    for c in range(NCORES):
        o = np.asarray(outs_per_core[c]).astype(np.float32)
        for g in range(NTCH):
            r0 = g * TCH + c * rows
            full[r0:r0 + rows] = o[g]
    return full.reshape(1, S, HIDDEN)


def run(inputs: dict, trace: bool = False):
    """Run on the 8 NeuronCores; returns (full_output, BassKernelResults)."""
    nc = build_program()
    in_maps = make_in_maps(**inputs)
    res = run_bass_kernel_spmd(nc, in_maps, core_ids=list(range(NCORES)),
                               trace=trace)
    full = assemble_output([res.results[c]["out"] for c in range(NCORES)])
    return full, res


def kernel(**inputs) -> np.ndarray:
    out, _ = run(inputs)
    return out


# revision 22
# speedup vs baseline: 1.1312x; 1.1312x over previous
# Mistral sliding-window attention (B=1, S=2048, H=4096, 32 q heads / 8 kv
# heads, window 4096 -> plain causal at this S) on 8 Trainium2 NeuronCores.
#
# Sharding: tensor-parallel over heads. Core c owns q heads 4c..4c+3 and kv
# head c; hidden_states replicated (host-transposed to [H, S] bf16).
#
# v3 design (dense-PE pipeline, bf16 data, stationary-reuse everywhere):
# the PE pays ~50ns extra per matmul whenever the stationary operand
# changes, so every loop is ordered to run back-to-back matmuls with the
# same stationary:
# - Phase A (QKV projection): chunk PAIRS - each weight slice w[k][:, m] is
#   stationary for two matmuls (token chunks 2cp and 2cp+1) accumulating in
#   [128, 2(chunk), 512] psum tags; m-halves {q0,q1,q2} / {q3,k,v} keep the
#   pair within 6 psum banks. x is streamed twice (DMA has headroom). ACT
#   copies psum->bf16 staging, RoPE on DVE during the next half, V tiles
#   PE-transposed off the critical path.
# - Phase B attention: per chunk c and HEAD PAIR, waves over kv tiles j:
#   kT[j] is stationary for both heads' score matmuls, V[j] for both PV
#   matmuls, ones for both denominator matmuls; exp on ACT in [128,2,512]
#   pt tiles (bf16), causal staircase on GpSimd, PV lag-6 queue so the PE
#   never waits on exp or the head-boundary normalize.
# - o_proj per token block g: column-chunk PAIRS - each ao tile is
#   stationary for two matmuls (columns 2ccp, 2ccp+1); all 4 heads
#   accumulate in PSUM across all 8 banks; outputs cast to bf16 (ACT+DVE
#   alternating), DMA'd to per-block partial tensors; ReduceScatter(add)
#   per block overlaps the next chunk's attention; final out copies ride
#   the gpsimd queue behind their RS.

from contextlib import ExitStack

import numpy as np
import ml_dtypes

import concourse.bacc as bacc
import concourse.bass as bass
import concourse.mybir as mybir
import concourse.tile as tile
from concourse.bass_utils import run_bass_kernel_spmd
from concourse.masks import make_identity

HIDDEN = 4096
NH = 32
NKV = 8
HD = 128
THETA = 10000.0
S = 2048
NCORES = 8

QH = NH // NCORES          # 4 q heads per core
DQ = QH * HD               # 512 (per-core q/attn width)
DOUT = DQ + 2 * HD         # 768 = q heads + k + v projection width
KT = HIDDEN // 128         # 32 contraction tiles
KG = 8                     # k-tiles per x DMA / inner k-group
TCH = 512                  # token chunk (matmul moving dim)
NTCH = S // TCH            # 4
KVT = S // 128             # 16 kv tiles
SCALE = 1.0 / float(np.sqrt(HD))

F32 = mybir.dt.float32
BF16 = mybir.dt.bfloat16
EXP = mybir.ActivationFunctionType.Exp
NPBF16 = ml_dtypes.bfloat16


def _rope(nc, rp, stg_half, qdst, cs2, sn2):
    """RoPE one [128, 512] head-tile: stg (bf16 SBUF) -> qdst (bf16 SBUF).

    qdst = stg*cs2 + rotate_half(stg)*sn2, with cs2 = [cos; cos] and
    sn2 = [-sin; sin] stacked on 128 partitions (host-precomputed), so all
    DVE ops are partition-aligned; the rotate is two SBUF->SBUF DMAs.
    """
    b = rp.tile([128, TCH], BF16, name="rope_b")
    nc.sync.dma_start(out=b[0:64, :], in_=stg_half[64:128, :])
    nc.sync.dma_start(out=b[64:128, :], in_=stg_half[0:64, :])
    ta = rp.tile([128, TCH], BF16, name="rope_t")
    tb = rp.tile([128, TCH], BF16, name="rope_u")
    nc.vector.tensor_mul(ta, stg_half, cs2)
    nc.vector.tensor_mul(tb, b, sn2)
    nc.vector.tensor_add(qdst, ta, tb)


def build_kernel_body(ctx: ExitStack, tc: tile.TileContext, outs, ins):
    nc = tc.nc
    xT, wqkv, ow, cos_t, sin_t, stair = (
        ins["xT"], ins["wqkv"], ins["ow"], ins["cos_t"], ins["sin_t"], ins["stair"],
    )
    out = outs["out"]

    # one partial tensor per token block so RS(g) never false-serializes
    # against the o_proj writes of block g+1
    partials = [nc.dram_tensor(f"partial{g}", [TCH, HIDDEN], BF16).ap()
                for g in range(NTCH)]
    rs_out = nc.dram_tensor("rs_out", [NTCH, S // NTCH // NCORES, HIDDEN],
                            BF16).ap()
    p3s = [p.rearrange("(b p) d -> p b d", p=128) for p in partials]

    singles = ctx.enter_context(tc.tile_pool(name="singles", bufs=1))
    # persistent bf16 state
    qT = singles.tile([128, QH, S], BF16)     # roped q, head h -> qT[:, h, :]
    kT = singles.tile([128, S], BF16)         # roped k
    V = singles.tile([128, KVT, HD], BF16)    # V[:, j, :] = [tok 128, d 128]
    ao = singles.tile([128, QH, S], BF16)     # attention out per head
    ones_sb = singles.tile([128, 128], BF16)
    ident_sb = singles.tile([128, 128], BF16)
    stair_sb = singles.tile([128, 896], BF16)
    cos_sb = singles.tile([128, S], BF16)
    sin_sb = singles.tile([128, S], BF16)
    ow_sb = singles.tile([128, QH, HIDDEN], BF16)   # o_w rows, d on partition

    # shared PSUM tags: four [128, 2, 512] f32 accumulators (8 banks)
    psh = ctx.enter_context(tc.tile_pool(name="psh", bufs=1, space="PSUM"))

    def T(i):
        return psh.tile([128, 2, TCH], F32, name=f"T{i}", tag=f"T{i}")

    # ---- DMA front matter ------------------------------------------------
    wq3 = wqkv.rearrange("(k p) d -> p k d", p=128)
    x3 = xT.rearrange("(k p) s -> p k s", p=128)

    with (
        tc.tile_pool(name="wq", bufs=1) as wp,
        tc.tile_pool(name="xt", bufs=6) as xp,
        tc.tile_pool(name="stg", bufs=2) as sp,
        tc.tile_pool(name="rope", bufs=3) as rp,
        tc.tile_pool(name="pva", bufs=1, space="PSUM") as ppv,
    ):
        # chunk-pair 0 supply: x tiles for chunks 0 and 1 interleaved with
        # the weight k-groups so k-ordered consumption is never starved
        w_sb = [wp.tile([128, DOUT], BF16, name=f"w{k}", tag=f"w{k}")
                for k in range(KT)]
        xg_cp0 = [[None] * (KT // KG) for _ in range(2)]
        for kg in range(KT // KG):
            for cc in range(2):
                xg = xp.tile([128, KG, TCH], BF16, name="xg")
                if kg == 0 and cc == 0:
                    nc.sync.dma_start(out=xg[:, 0:2, :], in_=x3[:, 0:2, 0:TCH])
                    nc.sync.dma_start(out=xg[:, 2:KG, :],
                                      in_=x3[:, 2:KG, 0:TCH])
                else:
                    nc.sync.dma_start(
                        out=xg, in_=x3[:, kg * KG:(kg + 1) * KG,
                                       cc * TCH:(cc + 1) * TCH])
                xg_cp0[cc][kg] = xg
            for k in range(kg * KG, (kg + 1) * KG):
                nc.sync.dma_start(out=w_sb[k], in_=wq3[:, k, :])
        nc.sync.dma_start(out=cos_sb, in_=cos_t)
        nc.sync.dma_start(out=sin_sb, in_=sin_t)
        nc.sync.dma_start(out=stair_sb, in_=stair)
        nc.vector.memset(ones_sb, 1.0)
        make_identity(nc, ident_sb)

        # ---- phase A: QKV projection (chunk pairs) + RoPE ----------------
        # queue of (kind, args) work from finished halves, drained into the
        # PE/ACT/DVE stream while later halves run
        pend = []

        def drain_pend(budget):
            n = 0
            while pend and n < budget:
                kind, args = pend.pop(0)
                if kind == "rope":
                    stg_half, dst, cs, sn = args
                    _rope(nc, rp, stg_half, dst, cs, sn)
                else:  # V transpose (PE, cheap, late-emitted)
                    stg_half, j = args
                    pvt = ppv.tile([128, 128], BF16, name="pv", tag="pv")
                    nc.tensor.transpose(pvt, stg_half, ident_sb)
                    nc.scalar.copy(out=V[:, j, :], in_=pvt)
                n += 1

        for cp in range(2):
            c0, c1 = 2 * cp, 2 * cp + 1
            for half in range(2):
                pss = [T(i) for i in range(3)]
                for kg in range(KT // KG):
                    xgs = []
                    for ci, c in enumerate((c0, c1)):
                        if cp == 0 and half == 0:
                            xg = xg_cp0[ci][kg]
                        else:
                            xg = xp.tile([128, KG, TCH], BF16, name="xg")
                            nc.sync.dma_start(
                                out=xg, in_=x3[:, kg * KG:(kg + 1) * KG,
                                               c * TCH:(c + 1) * TCH])
                        xgs.append(xg)
                    for ki in range(KG):
                        k = kg * KG + ki
                        for mi in range(3):
                            m = 3 * half + mi
                            for ci in range(2):
                                nc.tensor.matmul(
                                    pss[mi][:, ci, :],
                                    lhsT=w_sb[k][:, m * 128:(m + 1) * 128],
                                    rhs=xgs[ci][:, ki, :],
                                    start=(k == 0), stop=(k == KT - 1),
                                )
                    drain_pend(3 if kg > 0 else 0)
                # psum -> bf16 staging; split across ACT and DVE so the
                # tags free within ~1.5us for the next half
                stgs = []
                for mi in range(3):
                    stg = sp.tile([128, 2, TCH], BF16, name=f"stg{mi}")
                    if mi == 1:
                        nc.vector.tensor_copy(stg, pss[mi])
                    else:
                        nc.scalar.copy(out=stg, in_=pss[mi])
                    stgs.append(stg)
                # queue rope / V-transpose work for this half
                for ci, c in enumerate((c0, c1)):
                    lo = c * TCH
                    cs = cos_sb[:, lo:lo + TCH]
                    sn = sin_sb[:, lo:lo + TCH]
                    if half == 0:   # q heads 0,1,2
                        for mi in range(3):
                            pend.append(("rope", (stgs[mi][:, ci, :],
                                                  qT[:, mi, lo:lo + TCH],
                                                  cs, sn)))
                    else:           # q head 3, k, v
                        pend.append(("rope", (stgs[0][:, ci, :],
                                              qT[:, 3, lo:lo + TCH], cs, sn)))
                        pend.append(("rope", (stgs[1][:, ci, :],
                                              kT[:, lo:lo + TCH], cs, sn)))
                        for t in range(4):
                            j = 4 * c + t
                            pend.append(("vt", (
                                stgs[2][:, ci, t * 128:(t + 1) * 128], j)))
            if cp == 0:
                # o_proj weights: 4.2MB, queued after pair-0 supply so it
                # loads during pair 1 without starving the projection
                nc.sync.dma_start(out=ow_sb, in_=ow)
        drain_pend(10 ** 9)

    # ---- phase B: attention + o_proj + ReduceScatter ---------------------
    with (
        tc.tile_pool(name="pt", bufs=8) as ptp,
        tc.tile_pool(name="nrm", bufs=2) as nrmp,
        tc.tile_pool(name="ost", bufs=8) as ostp,
        tc.tile_pool(name="psb", bufs=1, space="PSUM") as psb,
    ):
        def T3():
            return psb.tile([128, 2, TCH], F32, name="T3", tag="T3")
        def attention_hpair(hp, c, pvq):
            """Attention for heads (2hp, 2hp+1), q chunk c. Score/PV/ones
            stationaries (kT[j], V[j], ones) are each shared by the two
            heads' back-to-back matmuls."""
            h0, h1 = 2 * hp, 2 * hp + 1
            jmax = 4 * c + 3
            tpo = T(2)   # po planes for the two heads
            tps = T3()   # ps planes for the two heads
            q0 = qT[:, h0, c * TCH:(c + 1) * TCH]
            q1 = qT[:, h1, c * TCH:(c + 1) * TCH]

            def pv_wave(args):
                pt, j = args
                nc.tensor.matmul(tpo[:, 0, :], lhsT=V[:, j, :],
                                 rhs=pt[:, 0, :],
                                 start=(j == 0), stop=(j == jmax))
                nc.tensor.matmul(tpo[:, 1, :], lhsT=V[:, j, :],
                                 rhs=pt[:, 1, :],
                                 start=(j == 0), stop=(j == jmax))
                nc.tensor.matmul(tps[:, 0, :], lhsT=ones_sb,
                                 rhs=pt[:, 0, :],
                                 start=(j == 0), stop=(j == jmax))
                nc.tensor.matmul(tps[:, 1, :], lhsT=ones_sb,
                                 rhs=pt[:, 1, :],
                                 start=(j == 0), stop=(j == jmax))

            for j in range(jmax + 1):
                sct = T(j % 2)
                nc.tensor.matmul(sct[:, 0, :],
                                 lhsT=kT[:, j * 128:(j + 1) * 128],
                                 rhs=q0, start=True, stop=True)
                nc.tensor.matmul(sct[:, 1, :],
                                 lhsT=kT[:, j * 128:(j + 1) * 128],
                                 rhs=q1, start=True, stop=True)
                pt = ptp.tile([128, 2, TCH], BF16, name="pt")
                nc.scalar.activation(pt, sct, EXP, scale=SCALE)
                rdiag = j - 4 * c
                if rdiag >= 0:  # tile touches the causal diagonal
                    off = 384 - rdiag * 128
                    for i in range(2):
                        nc.vector.tensor_mul(
                            pt[:, i, :], pt[:, i, :],
                            stair_sb[:, off:off + TCH])
                pvq.append((pv_wave, (pt, j), None))
                if len(pvq) > 6:
                    fn, args, fin = pvq.pop(0)
                    fn(args)
                    if fin is not None:
                        fin()

            def finalize(hp=hp, c=c, tpo=tpo, tps=tps):
                rec = nrmp.tile([128, 2, TCH], F32, name="rec")
                nc.vector.reciprocal(rec, tps)
                for i, h in enumerate((2 * hp, 2 * hp + 1)):
                    nc.vector.tensor_mul(
                        ao[:, h, c * TCH:(c + 1) * TCH],
                        tpo[:, i, :], rec[:, i, :])
            fn, args, fin = pvq[-1]
            pvq[-1] = (fn, args, finalize)

        def drain_pvq(pvq):
            while pvq:
                fn, args, fin = pvq.pop(0)
                fn(args)
                if fin is not None:
                    fin()

        def oproj_chunk(g):
            """partials[g] = local 4-head o_proj for tokens [g*512,(g+1)*512).
            Column pairs: each ao tile is stationary for two matmuls."""
            for ccp in range(HIDDEN // TCH // 2):
                cA, cB = 2 * ccp, 2 * ccp + 1
                acc = [T(0), T(1), T(2), T3()]  # [ccA: t01,t23][ccB: t01,t23]
                for h in range(QH):
                    for t in range(4):
                        lhs = ao[:, h, g * TCH + t * 128:g * TCH + (t + 1) * 128]
                        nc.tensor.matmul(
                            acc[t // 2][:, t % 2, :], lhsT=lhs,
                            rhs=ow_sb[:, h, cA * TCH:(cA + 1) * TCH],
                            start=(h == 0), stop=(h == QH - 1))
                        nc.tensor.matmul(
                            acc[2 + t // 2][:, t % 2, :], lhsT=lhs,
                            rhs=ow_sb[:, h, cB * TCH:(cB + 1) * TCH],
                            start=(h == 0), stop=(h == QH - 1))
                for q in range(4):
                    cc = cA if q < 2 else cB
                    ost = ostp.tile([128, 2, TCH], BF16, name="ost")
                    if q % 2 == 0:
                        nc.scalar.copy(out=ost, in_=acc[q])
                    else:
                        nc.vector.tensor_copy(ost, acc[q])
                    nc.sync.dma_start(
                        out=p3s[g][:, 2 * (q % 2):2 * (q % 2) + 2,
                                   cc * TCH:(cc + 1) * TCH],
                        in_=ost)

        for c in range(NTCH):
            pvq = []
            for hp in range(2):
                attention_hpair(hp, c, pvq)
            drain_pvq(pvq)
            oproj_chunk(c)
            nc.gpsimd.collective_compute(
                "ReduceScatter",
                mybir.AluOpType.add,
                ins=[partials[c][:, :]],
                outs=[rs_out[c]],
                replica_groups=[list(range(NCORES))],
            )
            # on the gpsimd queue (behind RS(c)) so it cannot head-of-line
            # block the sync-queue tile DMAs
            nc.gpsimd.dma_start(out=out[c], in_=rs_out[c])


_NC_CACHE = None


def build_program():
    global _NC_CACHE
    if _NC_CACHE is not None:
        return _NC_CACHE
    nc = bacc.Bacc("TRN2", target_bir_lowering=False, debug=False,
                   num_devices=NCORES)
    ins = {
        "xT": nc.dram_tensor("xT", [HIDDEN, S], BF16, kind="ExternalInput").ap(),
        "wqkv": nc.dram_tensor("wqkv", [HIDDEN, DOUT], BF16,
                               kind="ExternalInput").ap(),
        "ow": nc.dram_tensor("ow", [128, QH, HIDDEN], BF16,
                             kind="ExternalInput").ap(),
        "cos_t": nc.dram_tensor("cos_t", [128, S], BF16,
                                kind="ExternalInput").ap(),
        "sin_t": nc.dram_tensor("sin_t", [128, S], BF16,
                                kind="ExternalInput").ap(),
        "stair": nc.dram_tensor("stair", [128, 896], BF16,
                                kind="ExternalInput").ap(),
    }
    outs = {"out": nc.dram_tensor(
        "out", [NTCH, S // NTCH // NCORES, HIDDEN], BF16,
        kind="ExternalOutput").ap()}
    with tile.TileContext(nc) as tc:
        with ExitStack() as ctx:
            build_kernel_body(ctx, tc, outs, ins)
    nc.compile()
    _NC_CACHE = nc
    return nc


def make_in_maps(hidden_states, position_ids, q_w, k_w, v_w, o_w):
    x = np.asarray(hidden_states, dtype=np.float32).reshape(S, HIDDEN)
    xT = np.ascontiguousarray(x.T).astype(NPBF16)
    pos = np.asarray(position_ids).reshape(S).astype(np.float64)
    inv = 1.0 / (THETA ** (np.arange(0, HD, 2, dtype=np.float64) / HD))
    fr = inv[:, None] * pos[None, :]                       # [64, S]
    cos_t = np.concatenate([np.cos(fr), np.cos(fr)], 0).astype(NPBF16)
    sin_t = np.concatenate([-np.sin(fr), np.sin(fr)], 0).astype(NPBF16)
    u = np.arange(896, dtype=np.int64)[None, :]
    kvi = np.arange(128, dtype=np.int64)[:, None]
    stair = ((u - kvi) >= 384).astype(NPBF16)              # [128, 896]

    q_w = np.asarray(q_w, dtype=np.float32)
    k_w = np.asarray(k_w, dtype=np.float32)
    v_w = np.asarray(v_w, dtype=np.float32)
    o_w = np.asarray(o_w, dtype=np.float32)

    in_maps = []
    for c in range(NCORES):
        wqkv = np.ascontiguousarray(np.concatenate(
            [q_w[:, c * DQ:(c + 1) * DQ],
             k_w[:, c * HD:(c + 1) * HD],
             v_w[:, c * HD:(c + 1) * HD]], axis=1)).astype(NPBF16)
        owc = np.ascontiguousarray(
            o_w[c * DQ:(c + 1) * DQ, :].reshape(QH, 128, HIDDEN)
            .transpose(1, 0, 2)).astype(NPBF16)
        in_maps.append({"xT": xT, "wqkv": wqkv, "ow": owc,
                        "cos_t": cos_t, "sin_t": sin_t, "stair": stair})
    return in_maps


def assemble_output(outs_per_core):
    """outs_per_core[c] = [NTCH, 64, HIDDEN] bf16; stitch to [1, S, HIDDEN]."""
    rows = S // NTCH // NCORES
    full = np.empty((S, HIDDEN), dtype=np.float32)
    for c in range(NCORES):
        o = np.asarray(outs_per_core[c]).astype(np.float32)
        for g in range(NTCH):
            r0 = g * TCH + c * rows
            full[r0:r0 + rows] = o[g]
    return full.reshape(1, S, HIDDEN)


def run(inputs: dict, trace: bool = False):
    """Run on the 8 NeuronCores; returns (full_output, BassKernelResults)."""
    nc = build_program()
    in_maps = make_in_maps(**inputs)
    res = run_bass_kernel_spmd(nc, in_maps, core_ids=list(range(NCORES)),
                               trace=trace)
    full = assemble_output([res.results[c]["out"] for c in range(NCORES)])
    return full, res


def kernel(**inputs) -> np.ndarray:
    out, _ = run(inputs)
    return out


# revision 24
# speedup vs baseline: 1.2092x; 1.0690x over previous
# Mistral sliding-window attention (B=1, S=2048, H=4096, 32 q heads / 8 kv
# heads, window 4096 -> plain causal at this S) on 8 Trainium2 NeuronCores.
#
# Sharding: tensor-parallel over heads. Core c owns q heads 4c..4c+3 and kv
# head c; hidden_states replicated (host-transposed to [H, S] bf16).
#
# v3 design (dense-PE pipeline, bf16 data, stationary-reuse everywhere):
# the PE pays ~50ns extra per matmul whenever the stationary operand
# changes, so every loop is ordered to run back-to-back matmuls with the
# same stationary:
# - Phase A (QKV projection): chunk PAIRS - each weight slice w[k][:, m] is
#   stationary for two matmuls (token chunks 2cp and 2cp+1) accumulating in
#   [128, 2(chunk), 512] psum tags; m-halves {q0,q1,q2} / {q3,k,v} keep the
#   pair within 6 psum banks. x is streamed twice (DMA has headroom). ACT
#   copies psum->bf16 staging, RoPE on DVE during the next half, V tiles
#   PE-transposed off the critical path.
# - Phase B attention: per chunk c and HEAD PAIR, waves over kv tiles j:
#   kT[j] is stationary for both heads' score matmuls, V[j] for both PV
#   matmuls, ones for both denominator matmuls; exp on ACT in [128,2,512]
#   pt tiles (bf16), causal staircase on GpSimd, PV lag-6 queue so the PE
#   never waits on exp or the head-boundary normalize.
# - o_proj per token block g: column-chunk PAIRS - each ao tile is
#   stationary for two matmuls (columns 2ccp, 2ccp+1); all 4 heads
#   accumulate in PSUM across all 8 banks; outputs cast to bf16 (ACT+DVE
#   alternating), DMA'd to per-block partial tensors; ReduceScatter(add)
#   per block overlaps the next chunk's attention; final out copies ride
#   the gpsimd queue behind their RS.

from contextlib import ExitStack

import numpy as np
import ml_dtypes

import concourse.bacc as bacc
import concourse.bass as bass
import concourse.mybir as mybir
import concourse.tile as tile
from concourse.bass_utils import run_bass_kernel_spmd
from concourse.masks import make_identity

HIDDEN = 4096
NH = 32
NKV = 8
HD = 128
THETA = 10000.0
S = 2048
NCORES = 8

QH = NH // NCORES          # 4 q heads per core
DQ = QH * HD               # 512 (per-core q/attn width)
DOUT = DQ + 2 * HD         # 768 = q heads + k + v projection width
KT = HIDDEN // 128         # 32 contraction tiles
KG = 8                     # k-tiles per x DMA / inner k-group
TCH = 512                  # token chunk (matmul moving dim)
NTCH = S // TCH            # 4
KVT = S // 128             # 16 kv tiles
SCALE = 1.0 / float(np.sqrt(HD))

F32 = mybir.dt.float32
BF16 = mybir.dt.bfloat16
EXP = mybir.ActivationFunctionType.Exp
NPBF16 = ml_dtypes.bfloat16


def _rope(nc, rp, stg_half, qdst, cs2, sn2):
    """RoPE one [128, 512] head-tile: stg (bf16 SBUF) -> qdst (bf16 SBUF).

    qdst = stg*cs2 + rotate_half(stg)*sn2, with cs2 = [cos; cos] and
    sn2 = [-sin; sin] stacked on 128 partitions (host-precomputed), so all
    DVE ops are partition-aligned; the rotate is two SBUF->SBUF DMAs.
    """
    b = rp.tile([128, TCH], BF16, name="rope_b")
    nc.sync.dma_start(out=b[0:64, :], in_=stg_half[64:128, :])
    nc.sync.dma_start(out=b[64:128, :], in_=stg_half[0:64, :])
    ta = rp.tile([128, TCH], BF16, name="rope_t")
    tb = rp.tile([128, TCH], BF16, name="rope_u")
    nc.vector.tensor_mul(ta, stg_half, cs2)
    nc.vector.tensor_mul(tb, b, sn2)
    nc.vector.tensor_add(qdst, ta, tb)


def build_kernel_body(ctx: ExitStack, tc: tile.TileContext, outs, ins):
    nc = tc.nc
    xT, wqkv, ow, cos_t, sin_t, stair = (
        ins["xT"], ins["wqkv"], ins["ow"], ins["cos_t"], ins["sin_t"], ins["stair"],
    )
    out = outs["out"]

    # one partial tensor per token block so RS(g) never false-serializes
    # against the o_proj writes of block g+1
    partials = [nc.dram_tensor(f"partial{g}", [TCH, HIDDEN], BF16).ap()
                for g in range(NTCH)]
    rs_out = nc.dram_tensor("rs_out", [NTCH, S // NTCH // NCORES, HIDDEN],
                            BF16).ap()
    p3s = [p.rearrange("(b p) d -> p b d", p=128) for p in partials]
    warm_in = nc.dram_tensor("warm_in", [NCORES * 8, 64], BF16).ap()
    warm_out = nc.dram_tensor("warm_out", [8, 64], BF16).ap()

    singles = ctx.enter_context(tc.tile_pool(name="singles", bufs=1))
    # persistent bf16 state
    qT = singles.tile([128, QH, S], BF16)     # roped q, head h -> qT[:, h, :]
    kT = singles.tile([128, S], BF16)         # roped k
    V = singles.tile([128, KVT, HD], BF16)    # V[:, j, :] = [tok 128, d 128]
    ao = singles.tile([128, QH, S], BF16)     # attention out per head
    ones_sb = singles.tile([128, 128], BF16)
    ident_sb = singles.tile([128, 128], BF16)
    stair_sb = singles.tile([128, 896], BF16)
    cos_sb = singles.tile([128, S], BF16)
    sin_sb = singles.tile([128, S], BF16)
    ow_sb = singles.tile([128, QH, HIDDEN], BF16)   # o_w rows, d on partition

    # shared PSUM tags: four [128, 2, 512] f32 accumulators (8 banks)
    psh = ctx.enter_context(tc.tile_pool(name="psh", bufs=1, space="PSUM"))

    def T(i):
        return psh.tile([128, 2, TCH], F32, name=f"T{i}", tag=f"T{i}")

    # ---- DMA front matter ------------------------------------------------
    wq3 = wqkv.rearrange("(k p) d -> p k d", p=128)
    x3 = xT.rearrange("(k p) s -> p k s", p=128)

    with (
        tc.tile_pool(name="wq", bufs=1) as wp,
        tc.tile_pool(name="xt", bufs=6) as xp,
        tc.tile_pool(name="stg", bufs=2) as sp,
        tc.tile_pool(name="rope", bufs=3) as rp,
        tc.tile_pool(name="pva", bufs=1, space="PSUM") as ppv,
    ):
        # chunk-pair 0 supply: x tiles for chunks 0 and 1 interleaved with
        # the weight k-groups so k-ordered consumption is never starved
        w_sb = [wp.tile([128, DOUT], BF16, name=f"w{k}", tag=f"w{k}")
                for k in range(KT)]
        xg_cp0 = [[None] * (KT // KG) for _ in range(2)]
        for kg in range(KT // KG):
            for cc in range(2):
                xg = xp.tile([128, KG, TCH], BF16, name="xg")
                if kg == 0 and cc == 0:
                    nc.sync.dma_start(out=xg[:, 0:2, :], in_=x3[:, 0:2, 0:TCH])
                    nc.sync.dma_start(out=xg[:, 2:KG, :],
                                      in_=x3[:, 2:KG, 0:TCH])
                else:
                    nc.sync.dma_start(
                        out=xg, in_=x3[:, kg * KG:(kg + 1) * KG,
                                       cc * TCH:(cc + 1) * TCH])
                xg_cp0[cc][kg] = xg
            for k in range(kg * KG, (kg + 1) * KG):
                nc.sync.dma_start(out=w_sb[k], in_=wq3[:, k, :])
        nc.sync.dma_start(out=cos_sb, in_=cos_t)
        nc.sync.dma_start(out=sin_sb, in_=sin_t)
        nc.sync.dma_start(out=stair_sb, in_=stair)
        nc.vector.memset(ones_sb, 1.0)
        make_identity(nc, ident_sb)
        warm_sb = wp.tile([NCORES * 8, 64], BF16)
        nc.vector.memset(warm_sb, 0.0)
        nc.sync.dma_start(out=warm_in[:, :], in_=warm_sb)
        nc.gpsimd.collective_compute(
            "ReduceScatter", mybir.AluOpType.add,
            ins=[warm_in[:, :]], outs=[warm_out[:, :]],
            replica_groups=[list(range(NCORES))],
        )

        # ---- phase A: QKV projection (chunk pairs) + RoPE ----------------
        # queue of (kind, args) work from finished halves, drained into the
        # PE/ACT/DVE stream while later halves run
        pend = []

        def drain_pend(budget):
            n = 0
            while pend and n < budget:
                kind, args = pend.pop(0)
                if kind == "rope":
                    stg_half, dst, cs, sn = args
                    _rope(nc, rp, stg_half, dst, cs, sn)
                else:  # V transpose (PE, cheap, late-emitted)
                    stg_half, j = args
                    pvt = ppv.tile([128, 128], BF16, name="pv", tag="pv")
                    nc.tensor.transpose(pvt, stg_half, ident_sb)
                    nc.scalar.copy(out=V[:, j, :], in_=pvt)
                n += 1

        for cp in range(2):
            c0, c1 = 2 * cp, 2 * cp + 1
            for half in range(2):
                pss = [T(i) for i in range(3)]
                for kg in range(KT // KG):
                    xgs = []
                    for ci, c in enumerate((c0, c1)):
                        if cp == 0 and half == 0:
                            xg = xg_cp0[ci][kg]
                        else:
                            xg = xp.tile([128, KG, TCH], BF16, name="xg")
                            nc.sync.dma_start(
                                out=xg, in_=x3[:, kg * KG:(kg + 1) * KG,
                                               c * TCH:(c + 1) * TCH])
                        xgs.append(xg)
                    for ki in range(KG):
                        k = kg * KG + ki
                        for mi in range(3):
                            m = 3 * half + mi
                            for ci in range(2):
                                nc.tensor.matmul(
                                    pss[mi][:, ci, :],
                                    lhsT=w_sb[k][:, m * 128:(m + 1) * 128],
                                    rhs=xgs[ci][:, ki, :],
                                    start=(k == 0), stop=(k == KT - 1),
                                )
                    drain_pend(3 if kg > 0 else 0)
                # psum -> bf16 staging; split across ACT and DVE so the
                # tags free within ~1.5us for the next half
                stgs = []
                for mi in range(3):
                    stg = sp.tile([128, 2, TCH], BF16, name=f"stg{mi}")
                    if mi == 1:
                        nc.vector.tensor_copy(stg, pss[mi])
                    else:
                        nc.scalar.copy(out=stg, in_=pss[mi])
                    stgs.append(stg)
                # queue rope / V-transpose work for this half
                for ci, c in enumerate((c0, c1)):
                    lo = c * TCH
                    cs = cos_sb[:, lo:lo + TCH]
                    sn = sin_sb[:, lo:lo + TCH]
                    if half == 0:   # q heads 0,1,2
                        for mi in range(3):
                            pend.append(("rope", (stgs[mi][:, ci, :],
                                                  qT[:, mi, lo:lo + TCH],
                                                  cs, sn)))
                    else:           # q head 3, k, v
                        pend.append(("rope", (stgs[0][:, ci, :],
                                              qT[:, 3, lo:lo + TCH], cs, sn)))
                        pend.append(("rope", (stgs[1][:, ci, :],
                                              kT[:, lo:lo + TCH], cs, sn)))
                        for t in range(4):
                            j = 4 * c + t
                            pend.append(("vt", (
                                stgs[2][:, ci, t * 128:(t + 1) * 128], j)))
            if cp == 0:
                # o_proj weights: 4.2MB, queued after pair-0 supply so it
                # loads during pair 1 without starving the projection
                nc.sync.dma_start(out=ow_sb, in_=ow)
        drain_pend(10 ** 9)

    # ---- phase B: attention + o_proj + ReduceScatter ---------------------
    with (
        tc.tile_pool(name="pt", bufs=8) as ptp,
        tc.tile_pool(name="nrm", bufs=2) as nrmp,
        tc.tile_pool(name="ost", bufs=8) as ostp,
        tc.tile_pool(name="psb", bufs=1, space="PSUM") as psb,
    ):
        def T3():
            return psb.tile([128, 2, TCH], F32, name="T3", tag="T3")
        def attention_hpair(hp, c, pvq):
            """Attention for heads (2hp, 2hp+1), q chunk c. Score/PV/ones
            stationaries (kT[j], V[j], ones) are each shared by the two
            heads' back-to-back matmuls."""
            h0, h1 = 2 * hp, 2 * hp + 1
            jmax = 4 * c + 3
            tpo = T(2)   # po planes for the two heads
            tps = T3()   # ps planes for the two heads
            q0 = qT[:, h0, c * TCH:(c + 1) * TCH]
            q1 = qT[:, h1, c * TCH:(c + 1) * TCH]

            def pv_wave(args):
                pta, ptb, j, zt = args   # waves j and j+1
                for pt, jj in ((pta, j), (ptb, j + 1)):
                    nc.tensor.matmul(tpo[:, 0, :], lhsT=V[:, jj, :],
                                     rhs=pt[:, 0, :],
                                     start=(jj == 0), stop=(jj == jmax))
                    nc.tensor.matmul(tpo[:, 1, :], lhsT=V[:, jj, :],
                                     rhs=pt[:, 1, :],
                                     start=(jj == 0), stop=(jj == jmax))
                for i in range(2):
                    nc.tensor.matmul(tps[:, i, :], lhsT=ones_sb,
                                     rhs=zt[:, i, :],
                                     start=(j == 0), stop=(j + 1 == jmax))

            for j in range(jmax + 1):
                sct = T(j % 2)
                nc.tensor.matmul(sct[:, 0, :],
                                 lhsT=kT[:, j * 128:(j + 1) * 128],
                                 rhs=q0, start=True, stop=True)
                nc.tensor.matmul(sct[:, 1, :],
                                 lhsT=kT[:, j * 128:(j + 1) * 128],
                                 rhs=q1, start=True, stop=True)
                pt = ptp.tile([128, 2, TCH], BF16, name="pt")
                nc.scalar.activation(pt, sct, EXP, scale=SCALE)
                rdiag = j - 4 * c
                if rdiag >= 0:  # tile touches the causal diagonal
                    off = 384 - rdiag * 128
                    for i in range(2):
                        nc.vector.tensor_mul(
                            pt[:, i, :], pt[:, i, :],
                            stair_sb[:, off:off + TCH])
                if j % 2 == 0:
                    pt_prev = pt
                else:
                    zt = ptp.tile([128, 2, TCH], BF16, name="zt")
                    nc.vector.tensor_add(zt, pt_prev, pt)
                    pvq.append((pv_wave, (pt_prev, pt, j - 1, zt), None))
                if len(pvq) > 3:
                    fn, args, fin = pvq.pop(0)
                    fn(args)
                    if fin is not None:
                        fin()

            def finalize(hp=hp, c=c, tpo=tpo, tps=tps):
                rec = nrmp.tile([128, 2, TCH], F32, name="rec")
                nc.vector.reciprocal(rec, tps)
                for i, h in enumerate((2 * hp, 2 * hp + 1)):
                    nc.vector.tensor_mul(
                        ao[:, h, c * TCH:(c + 1) * TCH],
                        tpo[:, i, :], rec[:, i, :])
            fn, args, fin = pvq[-1]
            pvq[-1] = (fn, args, finalize)

        def drain_pvq(pvq):
            while pvq:
                fn, args, fin = pvq.pop(0)
                fn(args)
                if fin is not None:
                    fin()

        def oproj_chunk(g):
            """partials[g] = local 4-head o_proj for tokens [g*512,(g+1)*512).
            Column pairs: each ao tile is stationary for two matmuls."""
            for ccp in range(HIDDEN // TCH // 2):
                cA, cB = 2 * ccp, 2 * ccp + 1
                acc = [T(0), T(1), T(2), T3()]  # [ccA: t01,t23][ccB: t01,t23]
                for h in range(QH):
                    for t in range(4):
                        lhs = ao[:, h, g * TCH + t * 128:g * TCH + (t + 1) * 128]
                        nc.tensor.matmul(
                            acc[t // 2][:, t % 2, :], lhsT=lhs,
                            rhs=ow_sb[:, h, cA * TCH:(cA + 1) * TCH],
                            start=(h == 0), stop=(h == QH - 1))
                        nc.tensor.matmul(
                            acc[2 + t // 2][:, t % 2, :], lhsT=lhs,
                            rhs=ow_sb[:, h, cB * TCH:(cB + 1) * TCH],
                            start=(h == 0), stop=(h == QH - 1))
                for q in range(4):
                    cc = cA if q < 2 else cB
                    ost = ostp.tile([128, 2, TCH], BF16, name="ost")
                    if q % 2 == 0:
                        nc.scalar.copy(out=ost, in_=acc[q])
                    else:
                        nc.vector.tensor_copy(ost, acc[q])
                    nc.sync.dma_start(
                        out=p3s[g][:, 2 * (q % 2):2 * (q % 2) + 2,
                                   cc * TCH:(cc + 1) * TCH],
                        in_=ost)

        for c in range(NTCH):
            pvq = []
            for hp in range(2):
                attention_hpair(hp, c, pvq)
            drain_pvq(pvq)
            oproj_chunk(c)
            nc.gpsimd.collective_compute(
                "ReduceScatter",
                mybir.AluOpType.add,
                ins=[partials[c][:, :]],
                outs=[rs_out[c]],
                replica_groups=[list(range(NCORES))],
            )
            # on the gpsimd queue (behind RS(c)) so it cannot head-of-line
            # block the sync-queue tile DMAs
            nc.gpsimd.dma_start(out=out[c], in_=rs_out[c])


_NC_CACHE = None


def build_program():
    global _NC_CACHE
    if _NC_CACHE is not None:
        return _NC_CACHE
    nc = bacc.Bacc("TRN2", target_bir_lowering=False, debug=False,
                   num_devices=NCORES)
    ins = {
        "xT": nc.dram_tensor("xT", [HIDDEN, S], BF16, kind="ExternalInput").ap(),
        "wqkv": nc.dram_tensor("wqkv", [HIDDEN, DOUT], BF16,
                               kind="ExternalInput").ap(),
        "ow": nc.dram_tensor("ow", [128, QH, HIDDEN], BF16,
                             kind="ExternalInput").ap(),
        "cos_t": nc.dram_tensor("cos_t", [128, S], BF16,
                                kind="ExternalInput").ap(),
        "sin_t": nc.dram_tensor("sin_t", [128, S], BF16,
                                kind="ExternalInput").ap(),
        "stair": nc.dram_tensor("stair", [128, 896], BF16,
                                kind="ExternalInput").ap(),
    }
    outs = {"out": nc.dram_tensor(
        "out", [NTCH, S // NTCH // NCORES, HIDDEN], BF16,
        kind="ExternalOutput").ap()}
    with tile.TileContext(nc) as tc:
        with ExitStack() as ctx:
            build_kernel_body(ctx, tc, outs, ins)
    nc.compile()
    _NC_CACHE = nc
    return nc


def make_in_maps(hidden_states, position_ids, q_w, k_w, v_w, o_w):
    x = np.asarray(hidden_states, dtype=np.float32).reshape(S, HIDDEN)
    xT = np.ascontiguousarray(x.T).astype(NPBF16)
    pos = np.asarray(position_ids).reshape(S).astype(np.float64)
    inv = 1.0 / (THETA ** (np.arange(0, HD, 2, dtype=np.float64) / HD))
    fr = inv[:, None] * pos[None, :]                       # [64, S]
    cos_t = np.concatenate([np.cos(fr), np.cos(fr)], 0).astype(NPBF16)
    sin_t = np.concatenate([-np.sin(fr), np.sin(fr)], 0).astype(NPBF16)
    u = np.arange(896, dtype=np.int64)[None, :]
    kvi = np.arange(128, dtype=np.int64)[:, None]
    stair = ((u - kvi) >= 384).astype(NPBF16)              # [128, 896]

    q_w = np.asarray(q_w, dtype=np.float32)
    k_w = np.asarray(k_w, dtype=np.float32)
    v_w = np.asarray(v_w, dtype=np.float32)
    o_w = np.asarray(o_w, dtype=np.float32)

    in_maps = []
    for c in range(NCORES):
        wqkv = np.ascontiguousarray(np.concatenate(
            [q_w[:, c * DQ:(c + 1) * DQ],
             k_w[:, c * HD:(c + 1) * HD],
             v_w[:, c * HD:(c + 1) * HD]], axis=1)).astype(NPBF16)
        owc = np.ascontiguousarray(
            o_w[c * DQ:(c + 1) * DQ, :].reshape(QH, 128, HIDDEN)
            .transpose(1, 0, 2)).astype(NPBF16)
        in_maps.append({"xT": xT, "wqkv": wqkv, "ow": owc,
                        "cos_t": cos_t, "sin_t": sin_t, "stair": stair})
    return in_maps


def assemble_output(outs_per_core):
    """outs_per_core[c] = [NTCH, 64, HIDDEN] bf16; stitch to [1, S, HIDDEN]."""
    rows = S // NTCH // NCORES
    full = np.empty((S, HIDDEN), dtype=np.float32)
    for c in range(NCORES):
        o = np.asarray(outs_per_core[c]).astype(np.float32)
        for g in range(NTCH):
            r0 = g * TCH + c * rows
            full[r0:r0 + rows] = o[g]
    return full.reshape(1, S, HIDDEN)


def run(inputs: dict, trace: bool = False):
    """Run on the 8 NeuronCores; returns (full_output, BassKernelResults)."""
    nc = build_program()
    in_maps = make_in_maps(**inputs)
    res = run_bass_kernel_spmd(nc, in_maps, core_ids=list(range(NCORES)),
                               trace=trace)
    full = assemble_output([res.results[c]["out"] for c in range(NCORES)])
    return full, res


def kernel(**inputs) -> np.ndarray:
    out, _ = run(inputs)
    return out


# revision 27
# speedup vs baseline: 1.2311x; 1.0181x over previous
# Mistral sliding-window attention (B=1, S=2048, H=4096, 32 q heads / 8 kv
# heads, window 4096 -> plain causal at this S) on 8 Trainium2 NeuronCores.
#
# Sharding: tensor-parallel over heads. Core c owns q heads 4c..4c+3 and kv
# head c; hidden_states replicated (host-transposed to [H, S] bf16).
#
# v3 design (dense-PE pipeline, bf16 data, stationary-reuse everywhere):
# the PE pays ~50ns extra per matmul whenever the stationary operand
# changes, so every loop is ordered to run back-to-back matmuls with the
# same stationary:
# - Phase A (QKV projection): chunk PAIRS - each weight slice w[k][:, m] is
#   stationary for two matmuls (token chunks 2cp and 2cp+1) accumulating in
#   [128, 2(chunk), 512] psum tags; m-halves {q0,q1,q2} / {q3,k,v} keep the
#   pair within 6 psum banks. x is streamed twice (DMA has headroom). ACT
#   copies psum->bf16 staging, RoPE on DVE during the next half, V tiles
#   PE-transposed off the critical path.
# - Phase B attention: per chunk c and HEAD PAIR, waves over kv tiles j:
#   kT[j] is stationary for both heads' score matmuls, V[j] for both PV
#   matmuls, ones for both denominator matmuls; exp on ACT in [128,2,512]
#   pt tiles (bf16), causal staircase on GpSimd, PV lag-6 queue so the PE
#   never waits on exp or the head-boundary normalize.
# - o_proj per token block g: column-chunk PAIRS - each ao tile is
#   stationary for two matmuls (columns 2ccp, 2ccp+1); all 4 heads
#   accumulate in PSUM across all 8 banks; outputs cast to bf16 (ACT+DVE
#   alternating), DMA'd to per-block partial tensors; ReduceScatter(add)
#   per block overlaps the next chunk's attention; final out copies ride
#   the gpsimd queue behind their RS.

from contextlib import ExitStack

import numpy as np
import ml_dtypes

import concourse.bacc as bacc
import concourse.bass as bass
import concourse.mybir as mybir
import concourse.tile as tile
from concourse.bass_utils import run_bass_kernel_spmd
from concourse.masks import make_identity

HIDDEN = 4096
NH = 32
NKV = 8
HD = 128
THETA = 10000.0
S = 2048
NCORES = 8

QH = NH // NCORES          # 4 q heads per core
DQ = QH * HD               # 512 (per-core q/attn width)
DOUT = DQ + 2 * HD         # 768 = q heads + k + v projection width
KT = HIDDEN // 128         # 32 contraction tiles
KG = 8                     # k-tiles per x DMA / inner k-group
TCH = 512                  # token chunk (matmul moving dim)
NTCH = S // TCH            # 4
KVT = S // 128             # 16 kv tiles
SCALE = 1.0 / float(np.sqrt(HD))

F32 = mybir.dt.float32
BF16 = mybir.dt.bfloat16
FP16 = mybir.dt.float16
EXP = mybir.ActivationFunctionType.Exp
NPBF16 = ml_dtypes.bfloat16


def _rope(nc, rp, stg_half, qdst, cs2, sn2):
    """RoPE one [128, 512] head-tile: stg (bf16 SBUF) -> qdst (bf16 SBUF).

    qdst = stg*cs2 + rotate_half(stg)*sn2, with cs2 = [cos; cos] and
    sn2 = [-sin; sin] stacked on 128 partitions (host-precomputed), so all
    DVE ops are partition-aligned; the rotate is two SBUF->SBUF DMAs.
    """
    b = rp.tile([128, TCH], BF16, name="rope_b")
    nc.sync.dma_start(out=b[0:64, :], in_=stg_half[64:128, :])
    nc.sync.dma_start(out=b[64:128, :], in_=stg_half[0:64, :])
    ta = rp.tile([128, TCH], BF16, name="rope_t")
    tb = rp.tile([128, TCH], BF16, name="rope_u")
    nc.vector.tensor_mul(ta, stg_half, cs2)
    nc.vector.tensor_mul(tb, b, sn2)
    nc.vector.tensor_add(qdst, ta, tb)


def build_kernel_body(ctx: ExitStack, tc: tile.TileContext, outs, ins):
    nc = tc.nc
    xT, wqkv, ow, cos_t, sin_t, stair = (
        ins["xT"], ins["wqkv"], ins["ow"], ins["cos_t"], ins["sin_t"], ins["stair"],
    )
    out = outs["out"]

    # one partial tensor per token block so RS(g) never false-serializes
    # against the o_proj writes of block g+1
    partials = [nc.dram_tensor(f"partial{g}", [TCH, HIDDEN], BF16).ap()
                for g in range(NTCH)]
    rs_out = nc.dram_tensor("rs_out", [NTCH, S // NTCH // NCORES, HIDDEN],
                            BF16).ap()
    p3s = [p.rearrange("(b p) d -> p b d", p=128) for p in partials]
    warm_in = nc.dram_tensor("warm_in", [NCORES * 8, 64], BF16).ap()
    warm_out = nc.dram_tensor("warm_out", [8, 64], BF16).ap()

    singles = ctx.enter_context(tc.tile_pool(name="singles", bufs=1))
    # persistent bf16 state
    qT = singles.tile([128, QH, S], BF16)     # roped q, head h -> qT[:, h, :]
    kT = singles.tile([128, S], BF16)         # roped k
    V = singles.tile([128, KVT, HD], BF16)    # V[:, j, :] = [tok 128, d 128]
    ao = singles.tile([128, QH, S], BF16)     # attention out per head
    ones_sb = singles.tile([128, 128], FP16)
    bias_sb = singles.tile([128, 1], F32)
    ident_sb = singles.tile([128, 128], BF16)
    stair_sb = singles.tile([128, 896], BF16)
    cos_sb = singles.tile([128, S], BF16)
    sin_sb = singles.tile([128, S], BF16)
    ow_sb = singles.tile([128, QH, HIDDEN], BF16)   # o_w rows, d on partition

    # shared PSUM tags: four [128, 2, 512] f32 accumulators (8 banks)
    psh = ctx.enter_context(tc.tile_pool(name="psh", bufs=1, space="PSUM"))

    def T(i):
        return psh.tile([128, 2, TCH], F32, name=f"T{i}", tag=f"T{i}")

    # ---- DMA front matter ------------------------------------------------
    wq3 = wqkv.rearrange("(k p) d -> p k d", p=128)
    x3 = xT.rearrange("(k p) s -> p k s", p=128)

    with (
        tc.tile_pool(name="wq", bufs=1) as wp,
        tc.tile_pool(name="xt", bufs=6) as xp,
        tc.tile_pool(name="stg", bufs=2) as sp,
        tc.tile_pool(name="rope", bufs=3) as rp,
        tc.tile_pool(name="pva", bufs=1, space="PSUM") as ppv,
    ):
        # chunk-pair 0 supply: x tiles for chunks 0 and 1 interleaved with
        # the weight k-groups so k-ordered consumption is never starved
        w_sb = [wp.tile([128, DOUT], BF16, name=f"w{k}", tag=f"w{k}")
                for k in range(KT)]
        xg_cp0 = [[None] * (KT // KG) for _ in range(2)]
        for kg in range(KT // KG):
            for cc in range(2):
                xg = xp.tile([128, KG, TCH], BF16, name="xg")
                if kg == 0 and cc == 0:
                    nc.sync.dma_start(out=xg[:, 0:2, :], in_=x3[:, 0:2, 0:TCH])
                    nc.sync.dma_start(out=xg[:, 2:KG, :],
                                      in_=x3[:, 2:KG, 0:TCH])
                else:
                    nc.sync.dma_start(
                        out=xg, in_=x3[:, kg * KG:(kg + 1) * KG,
                                       cc * TCH:(cc + 1) * TCH])
                xg_cp0[cc][kg] = xg
            for k in range(kg * KG, (kg + 1) * KG):
                nc.sync.dma_start(out=w_sb[k], in_=wq3[:, k, :])
        nc.sync.dma_start(out=cos_sb, in_=cos_t)
        nc.sync.dma_start(out=sin_sb, in_=sin_t)
        nc.sync.dma_start(out=stair_sb, in_=stair)
        nc.vector.memset(ones_sb, 1.0)
        nc.vector.memset(bias_sb, -2.0)
        make_identity(nc, ident_sb)
        warm_sb = wp.tile([NCORES * 8, 64], BF16)
        nc.vector.memset(warm_sb, 0.0)
        nc.sync.dma_start(out=warm_in[:, :], in_=warm_sb)
        nc.gpsimd.collective_compute(
            "ReduceScatter", mybir.AluOpType.add,
            ins=[warm_in[:, :]], outs=[warm_out[:, :]],
            replica_groups=[list(range(NCORES))],
        )

        # ---- phase A: QKV projection (chunk pairs) + RoPE ----------------
        # queue of (kind, args) work from finished halves, drained into the
        # PE/ACT/DVE stream while later halves run
        pend = []

        def drain_pend(budget):
            n = 0
            while pend and n < budget:
                kind, args = pend.pop(0)
                if kind == "rope":
                    stg_half, dst, cs, sn = args
                    _rope(nc, rp, stg_half, dst, cs, sn)
                else:  # V transpose (PE, cheap, late-emitted)
                    stg_half, j = args
                    pvt = ppv.tile([128, 128], BF16, name="pv", tag="pv")
                    nc.tensor.transpose(pvt, stg_half, ident_sb)
                    nc.scalar.copy(out=V[:, j, :], in_=pvt)
                n += 1

        for cp in range(2):
            c0, c1 = 2 * cp, 2 * cp + 1
            for half in range(2):
                pss = [T(i) for i in range(3)]
                for kg in range(KT // KG):
                    xgs = []
                    for ci, c in enumerate((c0, c1)):
                        if cp == 0 and half == 0:
                            xg = xg_cp0[ci][kg]
                        else:
                            xg = xp.tile([128, KG, TCH], BF16, name="xg")
                            nc.sync.dma_start(
                                out=xg, in_=x3[:, kg * KG:(kg + 1) * KG,
                                               c * TCH:(c + 1) * TCH])
                        xgs.append(xg)
                    for ki in range(KG):
                        k = kg * KG + ki
                        for mi in range(3):
                            m = 3 * half + mi
                            for ci in range(2):
                                nc.tensor.matmul(
                                    pss[mi][:, ci, :],
                                    lhsT=w_sb[k][:, m * 128:(m + 1) * 128],
                                    rhs=xgs[ci][:, ki, :],
                                    start=(k == 0), stop=(k == KT - 1),
                                )
                    drain_pend(3 if kg > 0 else 0)
                # psum -> bf16 staging; split across ACT and DVE so the
                # tags free within ~1.5us for the next half
                stgs = []
                for mi in range(3):
                    stg = sp.tile([128, 2, TCH], BF16, name=f"stg{mi}")
                    if mi == 1:
                        nc.vector.tensor_copy(stg, pss[mi])
                    else:
                        nc.scalar.copy(out=stg, in_=pss[mi])
                    stgs.append(stg)
                # queue rope / V-transpose work for this half
                for ci, c in enumerate((c0, c1)):
                    lo = c * TCH
                    cs = cos_sb[:, lo:lo + TCH]
                    sn = sin_sb[:, lo:lo + TCH]
                    if half == 0:   # q heads 0,1,2
                        for mi in range(3):
                            pend.append(("rope", (stgs[mi][:, ci, :],
                                                  qT[:, mi, lo:lo + TCH],
                                                  cs, sn)))
                    else:           # q head 3, k, v
                        pend.append(("rope", (stgs[0][:, ci, :],
                                              qT[:, 3, lo:lo + TCH], cs, sn)))
                        pend.append(("rope", (stgs[1][:, ci, :],
                                              kT[:, lo:lo + TCH], cs, sn)))
                        for t in range(4):
                            j = 4 * c + t
                            pend.append(("vt", (
                                stgs[2][:, ci, t * 128:(t + 1) * 128], j)))
            if cp == 0:
                # o_proj weights: 4.2MB, queued after pair-0 supply so it
                # loads during pair 1 without starving the projection
                nc.sync.dma_start(out=ow_sb, in_=ow)
        drain_pend(10 ** 9)

    # ---- phase B: attention + o_proj + ReduceScatter ---------------------
    with (
        tc.tile_pool(name="pt", bufs=8) as ptp,
        tc.tile_pool(name="nrm", bufs=2) as nrmp,
        tc.tile_pool(name="ost", bufs=8) as ostp,
        tc.tile_pool(name="psb", bufs=1, space="PSUM") as psb,
    ):
        def T3():
            return psb.tile([128, 2, TCH], F32, name="T3", tag="T3")
        def attention_hpair(hp, c, pvq):
            """Attention for heads (2hp, 2hp+1), q chunk c. Score/PV/ones
            stationaries (kT[j], V[j], ones) are each shared by the two
            heads' back-to-back matmuls."""
            h0, h1 = 2 * hp, 2 * hp + 1
            jmax = 4 * c + 3
            tpo = T(2)   # po planes for the two heads
            tps = T3()   # ps planes for the two heads
            q0 = qT[:, h0, c * TCH:(c + 1) * TCH]
            q1 = qT[:, h1, c * TCH:(c + 1) * TCH]

            def pv_wave(args):
                pta, ptb, j, zt = args   # waves j and j+1
                for pt, jj in ((pta, j), (ptb, j + 1)):
                    nc.tensor.matmul(tpo[:, 0, :], lhsT=V[:, jj, :],
                                     rhs=pt[:, 0, :],
                                     start=(jj == 0), stop=(jj == jmax))
                    nc.tensor.matmul(tpo[:, 1, :], lhsT=V[:, jj, :],
                                     rhs=pt[:, 1, :],
                                     start=(jj == 0), stop=(jj == jmax))
                for i in range(2):
                    nc.tensor.matmul(tps[:, i, :], lhsT=ones_sb,
                                     rhs=zt[:, i, :],
                                     start=(j == 0), stop=(j + 1 == jmax))

            for j in range(jmax + 1):
                sct = T(j % 2)
                nc.tensor.matmul(sct[:, 0, :],
                                 lhsT=kT[:, j * 128:(j + 1) * 128],
                                 rhs=q0, start=True, stop=True)
                nc.tensor.matmul(sct[:, 1, :],
                                 lhsT=kT[:, j * 128:(j + 1) * 128],
                                 rhs=q1, start=True, stop=True)
                pt = ptp.tile([128, 2, TCH], BF16, name="pt")
                nc.scalar.activation(pt, sct, EXP, scale=SCALE, bias=bias_sb)
                rdiag = j - 4 * c
                if rdiag >= 0:  # tile touches the causal diagonal
                    off = 384 - rdiag * 128
                    for i in range(2):
                        nc.vector.tensor_mul(
                            pt[:, i, :], pt[:, i, :],
                            stair_sb[:, off:off + TCH])
                if j % 2 == 0:
                    pt_prev = pt
                else:
                    zt = ptp.tile([128, 2, TCH], FP16, name="zt")
                    nc.vector.tensor_add(zt, pt_prev, pt)
                    pvq.append((pv_wave, (pt_prev, pt, j - 1, zt), None))
                if len(pvq) > 3:
                    fn, args, fin = pvq.pop(0)
                    fn(args)
                    if fin is not None:
                        fin()

            def finalize(hp=hp, c=c, tpo=tpo, tps=tps):
                rec = nrmp.tile([128, 2, TCH], F32, name="rec")
                nc.vector.reciprocal(rec, tps)
                for i, h in enumerate((2 * hp, 2 * hp + 1)):
                    nc.vector.tensor_mul(
                        ao[:, h, c * TCH:(c + 1) * TCH],
                        tpo[:, i, :], rec[:, i, :])
            fn, args, fin = pvq[-1]
            pvq[-1] = (fn, args, finalize)

        def drain_pvq(pvq):
            while pvq:
                fn, args, fin = pvq.pop(0)
                fn(args)
                if fin is not None:
                    fin()

        def oproj_chunk(g):
            """partials[g] = local 4-head o_proj for tokens [g*512,(g+1)*512).
            Column pairs: each ao tile is stationary for two matmuls."""
            for ccp in range(HIDDEN // TCH // 2):
                cA, cB = 2 * ccp, 2 * ccp + 1
                acc = [T(0), T(1), T(2), T3()]  # [ccA: t01,t23][ccB: t01,t23]
                for h in range(QH):
                    for t in range(4):
                        lhs = ao[:, h, g * TCH + t * 128:g * TCH + (t + 1) * 128]
                        nc.tensor.matmul(
                            acc[t // 2][:, t % 2, :], lhsT=lhs,
                            rhs=ow_sb[:, h, cA * TCH:(cA + 1) * TCH],
                            start=(h == 0), stop=(h == QH - 1))
                        nc.tensor.matmul(
                            acc[2 + t // 2][:, t % 2, :], lhsT=lhs,
                            rhs=ow_sb[:, h, cB * TCH:(cB + 1) * TCH],
                            start=(h == 0), stop=(h == QH - 1))
                for q in range(4):
                    cc = cA if q < 2 else cB
                    ost = ostp.tile([128, 2, TCH], BF16, name="ost")
                    if q % 2 == 0:
                        nc.scalar.copy(out=ost, in_=acc[q])
                    else:
                        nc.vector.tensor_copy(ost, acc[q])
                    nc.sync.dma_start(
                        out=p3s[g][:, 2 * (q % 2):2 * (q % 2) + 2,
                                   cc * TCH:(cc + 1) * TCH],
                        in_=ost)

        for c in range(NTCH):
            pvq = []
            for hp in range(2):
                attention_hpair(hp, c, pvq)
            drain_pvq(pvq)
            oproj_chunk(c)
            nc.gpsimd.collective_compute(
                "ReduceScatter",
                mybir.AluOpType.add,
                ins=[partials[c][:, :]],
                outs=[rs_out[c]],
                replica_groups=[list(range(NCORES))],
            )
            # on the gpsimd queue (behind RS(c)) so it cannot head-of-line
            # block the sync-queue tile DMAs
            nc.gpsimd.dma_start(out=out[c], in_=rs_out[c])


_NC_CACHE = None


def build_program():
    global _NC_CACHE
    if _NC_CACHE is not None:
        return _NC_CACHE
    nc = bacc.Bacc("TRN2", target_bir_lowering=False, debug=False,
                   num_devices=NCORES)
    ins = {
        "xT": nc.dram_tensor("xT", [HIDDEN, S], BF16, kind="ExternalInput").ap(),
        "wqkv": nc.dram_tensor("wqkv", [HIDDEN, DOUT], BF16,
                               kind="ExternalInput").ap(),
        "ow": nc.dram_tensor("ow", [128, QH, HIDDEN], BF16,
                             kind="ExternalInput").ap(),
        "cos_t": nc.dram_tensor("cos_t", [128, S], BF16,
                                kind="ExternalInput").ap(),
        "sin_t": nc.dram_tensor("sin_t", [128, S], BF16,
                                kind="ExternalInput").ap(),
        "stair": nc.dram_tensor("stair", [128, 896], BF16,
                                kind="ExternalInput").ap(),
    }
    outs = {"out": nc.dram_tensor(
        "out", [NTCH, S // NTCH // NCORES, HIDDEN], BF16,
        kind="ExternalOutput").ap()}
    with tile.TileContext(nc) as tc:
        with ExitStack() as ctx:
            build_kernel_body(ctx, tc, outs, ins)
    nc.compile()
    _NC_CACHE = nc
    return nc


def make_in_maps(hidden_states, position_ids, q_w, k_w, v_w, o_w):
    x = np.asarray(hidden_states, dtype=np.float32).reshape(S, HIDDEN)
    xT = np.ascontiguousarray(x.T).astype(NPBF16)
    pos = np.asarray(position_ids).reshape(S).astype(np.float64)
    inv = 1.0 / (THETA ** (np.arange(0, HD, 2, dtype=np.float64) / HD))
    fr = inv[:, None] * pos[None, :]                       # [64, S]
    cos_t = np.concatenate([np.cos(fr), np.cos(fr)], 0).astype(NPBF16)
    sin_t = np.concatenate([-np.sin(fr), np.sin(fr)], 0).astype(NPBF16)
    u = np.arange(896, dtype=np.int64)[None, :]
    kvi = np.arange(128, dtype=np.int64)[:, None]
    stair = ((u - kvi) >= 384).astype(NPBF16)              # [128, 896]

    q_w = np.asarray(q_w, dtype=np.float32)
    k_w = np.asarray(k_w, dtype=np.float32)
    v_w = np.asarray(v_w, dtype=np.float32)
    o_w = np.asarray(o_w, dtype=np.float32)

    in_maps = []
    for c in range(NCORES):
        wqkv = np.ascontiguousarray(np.concatenate(
            [q_w[:, c * DQ:(c + 1) * DQ],
             k_w[:, c * HD:(c + 1) * HD],
             v_w[:, c * HD:(c + 1) * HD]], axis=1)).astype(NPBF16)
        owc = np.ascontiguousarray(
            o_w[c * DQ:(c + 1) * DQ, :].reshape(QH, 128, HIDDEN)
            .transpose(1, 0, 2)).astype(NPBF16)
        in_maps.append({"xT": xT, "wqkv": wqkv, "ow": owc,
                        "cos_t": cos_t, "sin_t": sin_t, "stair": stair})
    return in_maps


def assemble_output(outs_per_core):
    """outs_per_core[c] = [NTCH, 64, HIDDEN] bf16; stitch to [1, S, HIDDEN]."""
    rows = S // NTCH // NCORES
    full = np.empty((S, HIDDEN), dtype=np.float32)
    for c in range(NCORES):
        o = np.asarray(outs_per_core[c]).astype(np.float32)
        for g in range(NTCH):
            r0 = g * TCH + c * rows
            full[r0:r0 + rows] = o[g]
    return full.reshape(1, S, HIDDEN)


def run(inputs: dict, trace: bool = False):
    """Run on the 8 NeuronCores; returns (full_output, BassKernelResults)."""
    nc = build_program()
    in_maps = make_in_maps(**inputs)
    res = run_bass_kernel_spmd(nc, in_maps, core_ids=list(range(NCORES)),
                               trace=trace)
    full = assemble_output([res.results[c]["out"] for c in range(NCORES)])
    return full, res


def kernel(**inputs) -> np.ndarray:
    out, _ = run(inputs)
    return out
